# revision 20
# baseline (speedup 1.0000x reference)
"""Trainium2 Bass kernel for nn_DeformLayer (deformable conv block).

Sharding: data-parallel over batch, 1 sample per NeuronCore across 8 cores.

Per-core pipeline:
  offset conv (PE, bf16) -> PE-transpose om to position-major -> bilinear
  fields (DVE fp32) -> gather indices built ON-CHIP: idx = 64*y + x via
  16 selection matmuls (PE) into the dma_gather wrapped-16 layout
  (replicated across all 8 partition groups) -> one 2KB-chunk dma_gather
  per 128-position block from a host-doubled table xT2[p] =
  [ch(y,x), ch(y+1,x)] so one index fetches all 4 bilinear corners ->
  3-pass DVE combine (coef multiply with stride-0 channel broadcast,
  then yc/xs pair adds) -> dma_start_transpose to channel-major ->
  DCN matmul (PE) -> BN1+ReLU (ACT) -> 4-parity 2x2 deconv (PE)
  -> BN2+ReLU (ACT) -> out [256, 128, 128] fp32.
"""
import numpy as np
import ml_dtypes
from contextlib import ExitStack

import concourse.bass as bass
import concourse.tile as tile
from concourse import bacc, mybir
from concourse import bass_utils
from concourse.library_config import mlp

BF16 = ml_dtypes.bfloat16
F32 = mybir.dt.float32
BF = mybir.dt.bfloat16
I16 = mybir.dt.int16
I32 = mybir.dt.int32
AL = mybir.AluOpType
AF = mybir.ActivationFunctionType

EPS = 1e-5
H = W = 64
HW = H * W          # 4096
C = Co = 256
NHT = 16            # half-tiles of 256 positions
NBLK = 32           # 128-position blocks
PADHW = 66 * 66     # 4356

TAP0 = [(1, 0), (3, -1)]   # parity 0: (kh/kw, shift)
TAP1 = [(0, 1), (2, 0)]    # parity 1


def _ap(base, off, dims):
    return bass.AP(base.tensor, base.offset + off, [list(d) for d in dims])


def build_nc():
    nc = bacc.Bacc("TRN2", target_bir_lowering=False, debug=False,
                   num_devices=8, num_swdge_queues=4)

    d_xpad = nc.dram_tensor("xpad", [2, 128, PADHW], BF, kind="ExternalInput")
    d_xT2 = nc.dram_tensor("xT2", [(HW + 2) * 512], BF, kind="ExternalInput")
    d_FB = nc.dram_tensor("FB", [128, NBLK, 27], F32, kind="ExternalInput")
    d_woff = nc.dram_tensor("woff", [9, 2, 128, 27], BF, kind="ExternalInput")
    d_wdcn = nc.dram_tensor("wdcn", [128, 18, 256], BF, kind="ExternalInput")
    d_wup = nc.dram_tensor("wup", [128, 4, 8, 256], BF, kind="ExternalInput")
    d_bn1 = nc.dram_tensor("bn1", [2, 2, 128], F32, kind="ExternalInput")
    d_bn2 = nc.dram_tensor("bn2", [2, 2, 128], F32, kind="ExternalInput")
    d_id27 = nc.dram_tensor("id27", [27, 27], F32, kind="ExternalInput")
    d_sel = nc.dram_tensor("sel", [128, 8, 2, 128], BF, kind="ExternalInput")
    d_out = nc.dram_tensor("out", [256, 128, 128], F32, kind="ExternalOutput")

    with tile.TileContext(nc) as tc, ExitStack() as ctx:
        p_const = ctx.enter_context(tc.tile_pool(name="const", bufs=1))

        nc.gpsimd.load_library(mlp)

        # ---------------- persistent constants ----------------
        wdcn_sb = p_const.tile([128, 18, 256], BF)
        nc.sync.dma_start(wdcn_sb[:], d_wdcn.ap())
        wup_sb = p_const.tile([128, 4, 8, 256], BF)
        nc.sync.dma_start(wup_sb[:], d_wup.ap())
        bn1_sb = p_const.tile([128, 2, 2], F32)  # [o%128][s/b][ohalf]
        nc.sync.dma_start(
            bn1_sb[:], _ap(d_bn1.ap(), 0, [[1, 128], [256, 2], [128, 2]]))
        bn2_sb = p_const.tile([128, 2, 2], F32)
        nc.sync.dma_start(
            bn2_sb[:], _ap(d_bn2.ap(), 0, [[1, 128], [256, 2], [128, 2]]))
        # deconv input bands: [128(o%128), td(8), ohalf(2), 10*66] bf16, zeroed
        bands = p_const.tile([128, 8, 2, 660], BF)
        nc.vector.memset(bands[:], 0.0)
        # coefP [128, blk(32), k(9), xs(2), yc(2), dup(2)] bf16
        coefP = p_const.tile([128, NBLK * 72], BF)
        # gather indices, wrapped-16 replicated
        gidx = p_const.tile([128, NBLK, 72], I16)

        # ---------------- head phase (temp pools, freed before main loop) ---
        hctx = ExitStack()
        p_head = hctx.enter_context(tc.tile_pool(name="head", bufs=1))
        p_ftmp = hctx.enter_context(tc.tile_pool(name="ftmp", bufs=1))
        p_om = hctx.enter_context(tc.tile_pool(name="om", bufs=2))
        p_ps_om = hctx.enter_context(tc.tile_pool(name="psom", bufs=2, space="PSUM"))

        woff_sb = p_head.tile([128, 18, 27], BF)
        nc.sync.dma_start(
            woff_sb[:], _ap(d_woff.ap(), 0, [[27, 128], [128 * 27, 18], [1, 27]]))
        id27_sb = p_head.tile([27, 27], F32)
        nc.sync.dma_start(id27_sb[:], d_id27.ap())
        sel_sb = p_head.tile([128, 8, 2, 128], BF)
        nc.sync.dma_start(sel_sb[:], d_sel.ap())
        FB_sb = p_head.tile([128, NBLK, 27], F32)
        nc.sync.dma_start(FB_sb[:], d_FB.ap())
        xpad_sb = p_head.tile([128, 2, PADHW], BF)
        nc.sync.dma_start(
            xpad_sb[:],
            _ap(d_xpad.ap(), 0, [[PADHW, 128], [128 * PADHW, 2], [1, PADHW]]))

        # ---------------- offset conv + om transpose (per N-tile) ----------------
        omT = p_head.tile([128, NBLK, 27], F32)
        for nt in range(8):
            ps = p_ps_om.tile([27, 512], F32, tag="psom", name="psom")
            first = True
            for k in range(9):
                ky, kx = k // 3, k % 3
                for cb in range(2):
                    rhs = _ap(xpad_sb[:], cb * PADHW + (nt * 8 + ky) * 66 + kx,
                              [[2 * PADHW, 128], [66, 8], [1, 64]])
                    nc.tensor.matmul(ps[:], woff_sb[:, k * 2 + cb, :], rhs,
                                     start=first, stop=(k == 8 and cb == 1))
                    first = False
            om_nt = p_om.tile([27, 512], F32, tag="omnt", name="omnt")
            nc.scalar.copy(om_nt[:], ps[:])
            for i in range(4):
                pst = p_ps_om.tile([128, 27], F32, tag="pstr", name="pstr")
                nc.tensor.transpose(pst[:], om_nt[:, i * 128:(i + 1) * 128],
                                    id27_sb[:])
                nc.scalar.copy(omT[:, nt * 4 + i, :], pst[:])

        # ---------------- fields ----------------
        NF = NBLK * 9  # 288

        def f9(tt, j0):  # [128, NBLK, 9] view at channel offset j0
            return _ap(tt[:], j0, [[NBLK * 27, 128], [27, NBLK], [1, 9]])

        def ftile(tag, dt=F32):
            return p_ftmp.tile([128, NF], dt, tag=tag, name=tag)

        px = ftile("px"); py = ftile("py"); mask = ftile("mask")
        x0 = ftile("x0"); y0 = ftile("y0"); ti32 = ftile("i32tmp", I32)
        cy0 = ftile("cy0"); cy1 = ftile("cy1"); xp = ftile("xp"); yb = ftile("yb")
        ta = ftile("ta"); tb = ftile("tb"); tc_ = ftile("tc"); td = ftile("td")
        cx0 = ftile("cx0"); cx1 = ftile("cx1")
        yb_bf = p_ftmp.tile([128, NF], BF, tag="ybbf", name="ybbf")
        xp_bf = p_ftmp.tile([128, NF], BF, tag="xpbf", name="xpbf")

        nc.vector.tensor_tensor(px[:], f9(omT, 0), f9(FB_sb, 0), AL.add)
        nc.vector.tensor_tensor(py[:], f9(omT, 9), f9(FB_sb, 9), AL.add)
        nc.vector.tensor_tensor(mask[:], f9(omT, 18), f9(FB_sb, 18), AL.add)
        nc.scalar.activation(mask[:], mask[:], AF.Sigmoid)

        # floor, robust to cast rounding mode: f = cast(v); f -= (f > v)
        nc.vector.tensor_copy(ti32[:], px[:])
        nc.vector.tensor_copy(x0[:], ti32[:])
        nc.vector.tensor_tensor(ta[:], x0[:], px[:], AL.is_gt)
        nc.vector.tensor_tensor(x0[:], x0[:], ta[:], AL.subtract)
        nc.vector.tensor_copy(ti32[:], py[:])
        nc.vector.tensor_copy(y0[:], ti32[:])
        nc.vector.tensor_tensor(ta[:], y0[:], py[:], AL.is_gt)
        nc.vector.tensor_tensor(y0[:], y0[:], ta[:], AL.subtract)

        # px/py become fx/fy in place
        nc.vector.tensor_tensor(px[:], px[:], x0[:], AL.subtract)
        nc.vector.tensor_tensor(py[:], py[:], y0[:], AL.subtract)
        fx, fy = px, py

        # y slots: one chunk covers rows (yb, yb+1); yb = clip(y0,0,63).
        # wy0m = (1-fy)*vy0*mask ; wy1m = fy*vy1*mask
        # cy0 (row yb)   = wy0m + wy1m*[y0 == -1]
        # cy1 (row yb+1) = wy1m * [y0 >= 0]
        nc.vector.tensor_scalar(ta[:], y0[:], 0.0, None, AL.is_ge)
        nc.vector.tensor_scalar(tb[:], y0[:], 63.0, None, AL.is_le)
        nc.vector.tensor_tensor(ta[:], ta[:], tb[:], AL.mult)
        nc.vector.tensor_tensor(ta[:], ta[:], mask[:], AL.mult)
        nc.vector.tensor_scalar(tb[:], fy[:], -1.0, 1.0, AL.mult, AL.add)
        nc.vector.tensor_tensor(cy0[:], tb[:], ta[:], AL.mult)   # wy0m
        nc.vector.tensor_scalar(ta[:], y0[:], -1.0, None, AL.is_ge)
        nc.vector.tensor_scalar(tb[:], y0[:], 62.0, None, AL.is_le)
        nc.vector.tensor_tensor(ta[:], ta[:], tb[:], AL.mult)
        nc.vector.tensor_tensor(ta[:], ta[:], mask[:], AL.mult)
        nc.vector.tensor_tensor(cy1[:], fy[:], ta[:], AL.mult)   # wy1m
        nc.vector.tensor_scalar(ta[:], y0[:], -1.0, None, AL.is_equal)
        nc.vector.tensor_tensor(ta[:], ta[:], cy1[:], AL.mult)
        nc.vector.tensor_tensor(cy0[:], cy0[:], ta[:], AL.add)
        nc.vector.tensor_scalar(ta[:], y0[:], 0.0, None, AL.is_ge)
        nc.vector.tensor_tensor(cy1[:], cy1[:], ta[:], AL.mult)
        nc.vector.tensor_scalar(yb[:], y0[:], 0.0, 63.0, AL.max, AL.min)

        # x slots
        nc.vector.tensor_scalar(xp[:], x0[:], 0.0, 62.0, AL.max, AL.min)
        nc.vector.tensor_tensor(ta[:], x0[:], xp[:], AL.subtract)      # s
        nc.vector.tensor_scalar(tb[:], x0[:], 0.0, None, AL.is_ge)
        nc.vector.tensor_scalar(tc_[:], x0[:], 63.0, None, AL.is_le)
        nc.vector.tensor_tensor(tb[:], tb[:], tc_[:], AL.mult)         # vx0
        nc.vector.tensor_scalar(tc_[:], fx[:], -1.0, 1.0, AL.mult, AL.add)
        nc.vector.tensor_tensor(tc_[:], tc_[:], tb[:], AL.mult)        # wx0
        nc.vector.tensor_scalar(tb[:], x0[:], -1.0, None, AL.is_ge)
        nc.vector.tensor_scalar(td[:], x0[:], 62.0, None, AL.is_le)
        nc.vector.tensor_tensor(tb[:], tb[:], td[:], AL.mult)          # vx1
        nc.vector.tensor_tensor(td[:], fx[:], tb[:], AL.mult)          # wx1

        nc.vector.tensor_scalar(tb[:], ta[:], 0.0, None, AL.is_equal)
        nc.vector.tensor_tensor(cx0[:], tb[:], tc_[:], AL.mult)
        nc.vector.tensor_tensor(cx1[:], tb[:], td[:], AL.mult)
        nc.vector.tensor_scalar(tb[:], ta[:], -1.0, None, AL.is_equal)
        nc.vector.tensor_tensor(tb[:], tb[:], td[:], AL.mult)
        nc.vector.tensor_tensor(cx0[:], cx0[:], tb[:], AL.add)
        nc.vector.tensor_scalar(tb[:], ta[:], 1.0, None, AL.is_equal)
        nc.vector.tensor_tensor(tb[:], tb[:], tc_[:], AL.mult)
        nc.vector.tensor_tensor(cx1[:], cx1[:], tb[:], AL.add)

        # coefP: each coef stored twice (dup pairs) so pass1 can read
        # 4B-aligned step-1 pairs: col = blk*72 + (k*4 + xs*2 + yc)*2 + dup
        def coef_ap(xs, yc, dup):
            return _ap(coefP[:], (xs * 2 + yc) * 2 + dup,
                       [[NBLK * 72, 128], [72, NBLK], [8, 9]])

        for (cxv, cyv, xs, yc) in ((cx0, cy0, 0, 0), (cx0, cy1, 0, 1),
                                   (cx1, cy0, 1, 0), (cx1, cy1, 1, 1)):
            nc.vector.tensor_tensor(tb[:], cxv[:], cyv[:], AL.mult)
            nc.vector.tensor_copy(coef_ap(xs, yc, 0), tb[:])
            nc.vector.tensor_copy(coef_ap(xs, yc, 1), tb[:])

        # ---------------- on-chip gather index build ----------------
        # gidx [128, blk(32), k(9)*8+g] int16, idx = 64*yb + xp, wrapped-16
        # layout replicated across the 8 partition groups via selection
        # matmuls: out_g[i, c] = 64*yb[g*16 + i%16, c] + xp[g*16 + i%16, c].
        nc.vector.tensor_copy(yb_bf[:], yb[:])
        nc.vector.tensor_copy(xp_bf[:], xp[:])
        for g in range(8):
            psi = p_ps_om.tile([128, NF], F32, tag="psom", name="psidx")
            nc.tensor.matmul(psi[:], sel_sb[:, g, 0, :], yb_bf[:],
                             start=True, stop=False)
            nc.tensor.matmul(psi[:], sel_sb[:, g, 1, :], xp_bf[:],
                             start=False, stop=True)
            nc.vector.tensor_copy(
                _ap(gidx[:], g, [[NBLK * 72, 128], [72, NBLK], [8, 9]]),
                psi[:])

        # free head-phase SBUF/PSUM before the main loop
        hctx.close()

        p_G = ctx.enter_context(tc.tile_pool(name="G", bufs=5))
        p_gT = ctx.enter_context(tc.tile_pool(name="gT", bufs=2))
        p_gall = ctx.enter_context(tc.tile_pool(name="gall", bufs=2))
        p_ps_dcn = ctx.enter_context(tc.tile_pool(name="psdcn", bufs=2, space="PSUM"))
        p_ps_dc = ctx.enter_context(tc.tile_pool(name="psdc", bufs=2, space="PSUM"))
        p_outst = ctx.enter_context(tc.tile_pool(name="outst", bufs=2))

        xT2_src = _ap(d_xT2.ap(), 0, [[512, HW + 1], [1, 1024]])

        # ---------------- main loop over 128-position blocks ----------------
        for h in range(NHT):
            gall = p_gall.tile([128, 2, 18, 128], BF, tag="gall", name="gall")
            for pb in range(2):
                blk = h * 2 + pb
                G = p_G.tile([128, 9, 1024], BF, tag="G", name="G")
                nc.gpsimd.dma_gather(
                    G[:], xT2_src,
                    _ap(gidx[:], blk * 72, [[NBLK * 72, 128], [1, 72]]),
                    1152, 1152, 1024, elem_step=512,
                    queue_num=blk % 4, single_packet=False)
                # pass1 (in-place): G *= coef broadcast over channels.
                # Split into two ops for finer port-lock interleaving with
                # Q7 descgen.
                for kc0, nkc in ((0, 18), (18, 18)):
                    g_view = _ap(G[:], kc0 * 256,
                                 [[9 * 1024, 128], [256, nkc], [1, 256]])
                    c_view = _ap(coefP[:], blk * 72 + kc0 * 2,
                                 [[NBLK * 72, 128], [2, nkc], [0, 256]])
                    nc.vector.tensor_tensor(g_view, g_view, c_view, AL.mult)
                # pass2: yc pairs into yc0 slots
                ev = _ap(G[:], 0, [[9 * 1024, 128], [512, 18], [1, 256]])
                od = _ap(G[:], 256, [[9 * 1024, 128], [512, 18], [1, 256]])
                nc.vector.tensor_tensor(ev, ev, od, AL.add)
                # pass3: xs pairs -> gT [128, k(9), 256]
                gT = p_gT.tile([128, 9 * 256], BF, tag="gT", name="gT")
                p3o = _ap(gT[:], 0, [[9 * 256, 128], [256, 9], [1, 256]])
                p3a = _ap(G[:], 0, [[9 * 1024, 128], [1024, 9], [1, 256]])
                p3b = _ap(G[:], 512, [[9 * 1024, 128], [1024, 9], [1, 256]])
                nc.vector.tensor_tensor(p3o, p3a, p3b, AL.add)
                # gT holds 9 k-chunks (one pb) = [128, 2304]; contiguous dest
                nc.sync.dma_start_transpose(
                    _ap(gall[:], pb * 2304,
                        [[2 * 18 * 128, 128], [128, 18], [1, 128]]),
                    gT[:])
            # DCN matmul + BN1+ReLU into bands
            for ohalf in range(2):
                ps = p_ps_dcn.tile([128, 256], F32, tag="psdcn", name="psdcn")
                for j in range(18):
                    lhsT = _ap(wdcn_sb[:], j * 256 + ohalf * 128,
                               [[18 * 256, 128], [1, 128]])
                    rhs = _ap(gall[:], j * 128,
                              [[2 * 18 * 128, 128], [2304, 2], [1, 128]])
                    nc.tensor.matmul(ps[:], lhsT, rhs,
                                     start=(j == 0), stop=(j == 17))
                td0 = h // 2
                loc0 = 4 * (h % 2) + 1
                bb = bn1_sb[:, 1, ohalf:ohalf + 1]
                ss = bn1_sb[:, 0, ohalf:ohalf + 1]
                nc.scalar.activation(
                    _ap(bands[:], td0 * 1320 + ohalf * 660 + loc0 * 66 + 1,
                        [[8 * 2 * 660, 128], [66, 4], [1, 64]]),
                    ps[:], AF.Relu, bias=bb, scale=ss)
                if h % 2 == 0 and td0 > 0:
                    nc.scalar.activation(
                        _ap(bands[:], (td0 - 1) * 1320 + ohalf * 660 + 9 * 66 + 1,
                            [[8 * 2 * 660, 128], [1, 64]]),
                        ps[:, 0:64], AF.Relu, bias=bb, scale=ss)
                if h % 2 == 1 and td0 < 7:
                    nc.scalar.activation(
                        _ap(bands[:], (td0 + 1) * 1320 + ohalf * 660 + 1,
                            [[8 * 2 * 660, 128], [1, 64]]),
                        ps[:, 192:256], AF.Relu, bias=bb, scale=ss)

            # deconv for ready band
            td_ = None
            if h >= 2 and h % 2 == 0:
                td_ = h // 2 - 1
            elif h == NHT - 1:
                td_ = 7
            if td_ is None:
                continue
            for ohalf in range(2):
                outst = p_outst.tile([128, 2048], F32, tag="outst", name="outst")
                for par in range(4):
                    a, b_ = par // 2, par % 2
                    tap_y = TAP0 if a == 0 else TAP1
                    tap_x = TAP0 if b_ == 0 else TAP1
                    ps = p_ps_dc.tile([128, 512], F32, tag="psdc", name="psdc")
                    for j8 in range(8):
                        ti, tj, chalf = j8 // 4, (j8 // 2) % 2, j8 % 2
                        dr, ds = tap_y[ti][1], tap_x[tj][1]
                        lhsT = _ap(wup_sb[:],
                                   par * 8 * 256 + j8 * 256 + ohalf * 128,
                                   [[4 * 8 * 256, 128], [1, 128]])
                        rhs = _ap(bands[:],
                                  td_ * 1320 + chalf * 660 + (1 + dr) * 66 + 1 + ds,
                                  [[8 * 2 * 660, 128], [66, 8], [1, 64]])
                        nc.tensor.matmul(ps[:], lhsT, rhs,
                                         start=(j8 == 0), stop=(j8 == 7))
                    nc.scalar.activation(
                        _ap(outst[:], a * 128 + b_,
                            [[2048, 128], [256, 8], [2, 64]]),
                        ps[:], AF.Relu,
                        bias=bn2_sb[:, 1, ohalf:ohalf + 1], scale=bn2_sb[:, 0, ohalf:ohalf + 1])
                nc.sync.dma_start(
                    _ap(d_out.ap(), ohalf * 128 * 16384 + td_ * 16 * 128,
                        [[16384, 128], [1, 2048]]),
                    outst[:])

    nc.compile()
    return nc


# ---------------- host prep ----------------
def _prep_shared(inputs):
    w_off = np.asarray(inputs["w_off"], np.float32)
    b_off = np.asarray(inputs["b_off"], np.float32)
    w_dcn = np.asarray(inputs["w_dcn"], np.float32)
    w_up = np.asarray(inputs["w_up"], np.float32)

    woff = np.zeros((9, 2, 128, 27), np.float32)
    for k in range(9):
        for cb in range(2):
            woff[k, cb] = w_off[:, cb * 128:(cb + 1) * 128, k // 3, k % 3].T
    wdcn = np.zeros((128, 18, 256), np.float32)
    wd = w_dcn.reshape(Co, C, 9)
    for k in range(9):
        for chalf in range(2):
            wdcn[:, k * 2 + chalf, :] = wd[:, chalf * 128:(chalf + 1) * 128, k].T
    wup = np.zeros((128, 4, 8, 256), np.float32)
    for par in range(4):
        a, b_ = par // 2, par % 2
        tap_y = TAP0 if a == 0 else TAP1
        tap_x = TAP0 if b_ == 0 else TAP1
        for j8 in range(8):
            ti, tj, chalf = j8 // 4, (j8 // 2) % 2, j8 % 2
            kh, kw = tap_y[ti][0], tap_x[tj][0]
            # lhsT[p=c%128, o] = w_eff[o, c] = w_up[o, c, kh, kw]
            wup[:, par, j8, :] = w_up[:, chalf * 128:(chalf + 1) * 128, kh, kw].T

    ky = np.repeat(np.arange(3) - 1, 3).astype(np.float32)
    kx = np.tile(np.arange(3) - 1, 3).astype(np.float32)
    pos = np.arange(HW)
    hh = (pos // W).astype(np.float32)
    ww = (pos % W).astype(np.float32)
    FBp = np.zeros((HW, 27), np.float32)
    FBp[:, 0:9] = ww[:, None] + kx[None, :]
    FBp[:, 9:18] = hh[:, None] + ky[None, :]
    FBp += b_off[None, :]
    FB = np.ascontiguousarray(FBp.reshape(NBLK, 128, 27).transpose(1, 0, 2))

    # selection weights for the on-chip index fold:
    # sel[q, g, 0, i] = 64 if q == g*16 + i%16 ; sel[q, g, 1, i] = 1 if same
    sel = np.zeros((128, 8, 2, 128), np.float32)
    q = np.arange(128)
    i = np.arange(128)
    for g in range(8):
        m = (q[:, None] == g * 16 + (i[None, :] % 16))
        sel[:, g, 0, :] = 64.0 * m
        sel[:, g, 1, :] = 1.0 * m

    def bnfold(g, b, m, v):
        s = np.asarray(g) / np.sqrt(np.asarray(v) + EPS)
        return s.astype(np.float32), (np.asarray(b) - np.asarray(m) * s).astype(np.float32)

    s1, b1 = bnfold(inputs["bn1_g"], inputs["bn1_b"], inputs["bn1_m"], inputs["bn1_v"])
    s2, b2 = bnfold(inputs["bn2_g"], inputs["bn2_b"], inputs["bn2_m"], inputs["bn2_v"])
    bn1 = np.stack([s1.reshape(2, 128), b1.reshape(2, 128)])
    bn2 = np.stack([s2.reshape(2, 128), b2.reshape(2, 128)])

    return dict(
        woff=woff.astype(BF16), wdcn=wdcn.astype(BF16), wup=wup.astype(BF16),
        FB=FB.astype(np.float32), bn1=bn1.astype(np.float32),
        bn2=bn2.astype(np.float32), id27=np.eye(27, dtype=np.float32),
        sel=sel.astype(BF16),
    )


def _prep_sample(xb):
    xb = np.asarray(xb, np.float32)
    xpad = np.zeros((C, 66, 66), np.float32)
    xpad[:, 1:65, 1:65] = xb.reshape(C, 64, 64)
    xpad = xpad.reshape(2, 128, PADHW)
    # xT2[p] = [ch(y,x), ch(y+1,x)] for p = y*64+x; zeros for y+1 == 64
    xT = xb.reshape(C, HW).T          # [HW, 256]
    xT2 = np.zeros((HW + 2, 512), np.float32)
    xT2[:HW, 0:256] = xT
    xT2[:HW - 64, 256:512] = xT[64:]
    return dict(xpad=xpad.astype(BF16), xT2=xT2.reshape(-1).astype(BF16))


_NC_CACHE = {}
TRACE = False
LAST_RESULT = None


def kernel(**inputs):
    global LAST_RESULT
    if "nc" not in _NC_CACHE:
        _NC_CACHE["nc"] = build_nc()
    nc = _NC_CACHE["nc"]
    shared = _prep_shared(inputs)
    x = np.asarray(inputs["x"])
    in_maps = [dict(shared, **_prep_sample(x[b])) for b in range(x.shape[0])]
    res = bass_utils.run_bass_kernel_spmd(nc, in_maps, core_ids=list(range(8)),
                                          trace=TRACE)
    LAST_RESULT = res
    out = np.stack([res.results[b]["out"] for b in range(len(in_maps))])
    return out.astype(np.float32)


# revision 21
# speedup vs baseline: 1.1064x; 1.1064x over previous
"""Trainium2 Bass kernel for nn_DeformLayer (deformable conv block).

Sharding: data-parallel over batch, 1 sample per NeuronCore across 8 cores.

Per-core pipeline:
  offset conv (PE, bf16) -> PE-transpose om to position-major -> bilinear
  fields (DVE fp32) -> gather indices built ON-CHIP: idx = 64*y + x via
  16 selection matmuls (PE) into the dma_gather wrapped-16 layout
  (replicated across all 8 partition groups) -> one 2KB-chunk dma_gather
  per 128-position block from a host-doubled table xT2[p] =
  [ch(y,x), ch(y+1,x)] so one index fetches all 4 bilinear corners ->
  3-pass DVE combine (coef multiply with stride-0 channel broadcast,
  then yc/xs pair adds) -> dma_start_transpose to channel-major ->
  DCN matmul (PE) -> BN1+ReLU (ACT) -> 4-parity 2x2 deconv (PE)
  -> BN2+ReLU (ACT) -> out [256, 128, 128] fp32.
"""
import numpy as np
import ml_dtypes
from contextlib import ExitStack

import concourse.bass as bass
import concourse.tile as tile
from concourse import bacc, mybir
from concourse import bass_utils
from concourse.library_config import mlp

BF16 = ml_dtypes.bfloat16
F32 = mybir.dt.float32
BF = mybir.dt.bfloat16
I16 = mybir.dt.int16
I32 = mybir.dt.int32
AL = mybir.AluOpType
AF = mybir.ActivationFunctionType

EPS = 1e-5
H = W = 64
HW = H * W          # 4096
C = Co = 256
NHT = 16            # half-tiles of 256 positions
NBLK = 32           # 128-position blocks
PADHW = 66 * 66     # 4356

TAP0 = [(1, 0), (3, -1)]   # parity 0: (kh/kw, shift)
TAP1 = [(0, 1), (2, 0)]    # parity 1


def _ap(base, off, dims):
    return bass.AP(base.tensor, base.offset + off, [list(d) for d in dims])


def build_nc():
    nc = bacc.Bacc("TRN2", target_bir_lowering=False, debug=False,
                   num_devices=8, num_swdge_queues=4)

    d_xpad = nc.dram_tensor("xpad", [2, 128, PADHW], BF, kind="ExternalInput")
    d_xT2 = nc.dram_tensor("xT2", [(HW + 2) * 512], BF, kind="ExternalInput")
    d_FB = nc.dram_tensor("FB", [128, NBLK, 27], F32, kind="ExternalInput")
    d_woff = nc.dram_tensor("woff", [9, 2, 128, 27], BF, kind="ExternalInput")
    d_wdcn = nc.dram_tensor("wdcn", [128, 18, 256], BF, kind="ExternalInput")
    d_wup = nc.dram_tensor("wup", [128, 4, 8, 256], BF, kind="ExternalInput")
    d_bn1 = nc.dram_tensor("bn1", [2, 2, 128], F32, kind="ExternalInput")
    d_bn2 = nc.dram_tensor("bn2", [2, 2, 128], F32, kind="ExternalInput")
    d_id27 = nc.dram_tensor("id27", [27, 27], F32, kind="ExternalInput")
    d_sel = nc.dram_tensor("sel", [128, 8, 2, 128], BF, kind="ExternalInput")
    d_out = nc.dram_tensor("out", [256, 128, 128], F32, kind="ExternalOutput")

    with tile.TileContext(nc) as tc, ExitStack() as ctx:
        p_const = ctx.enter_context(tc.tile_pool(name="const", bufs=1))

        nc.gpsimd.load_library(mlp)

        # ---------------- persistent constants ----------------
        wdcn_sb = p_const.tile([128, 18, 256], BF)
        nc.sync.dma_start(wdcn_sb[:], d_wdcn.ap())
        wup_sb = p_const.tile([128, 4, 8, 256], BF)
        nc.sync.dma_start(wup_sb[:], d_wup.ap())
        bn1_sb = p_const.tile([128, 2, 2], F32)  # [o%128][s/b][ohalf]
        nc.sync.dma_start(
            bn1_sb[:], _ap(d_bn1.ap(), 0, [[1, 128], [256, 2], [128, 2]]))
        bn2_sb = p_const.tile([128, 2, 2], F32)
        nc.sync.dma_start(
            bn2_sb[:], _ap(d_bn2.ap(), 0, [[1, 128], [256, 2], [128, 2]]))
        # deconv input bands: [128(o%128), td(8), ohalf(2), 10*66] bf16, zeroed
        bands = p_const.tile([128, 8, 2, 660], BF)
        nc.vector.memset(bands[:], 0.0)
        # coefP [128, blk(32), k(9), xs(2), yc(2), dup(2)] bf16
        coefP = p_const.tile([128, NBLK * 72], BF)
        # gather indices, wrapped-16 replicated
        gidx = p_const.tile([128, NBLK, 72], I16)

        # ---------------- head phase (temp pools, freed before main loop) ---
        hctx = ExitStack()
        p_head = hctx.enter_context(tc.tile_pool(name="head", bufs=1))
        p_ftmp = hctx.enter_context(tc.tile_pool(name="ftmp", bufs=1))
        p_om = hctx.enter_context(tc.tile_pool(name="om", bufs=2))
        p_ps_om = hctx.enter_context(tc.tile_pool(name="psom", bufs=2, space="PSUM"))

        woff_sb = p_head.tile([128, 18, 27], BF)
        nc.sync.dma_start(
            woff_sb[:], _ap(d_woff.ap(), 0, [[27, 128], [128 * 27, 18], [1, 27]]))
        id27_sb = p_head.tile([27, 27], F32)
        nc.sync.dma_start(id27_sb[:], d_id27.ap())
        sel_sb = p_head.tile([128, 8, 2, 128], BF)
        nc.sync.dma_start(sel_sb[:], d_sel.ap())
        FB_sb = p_head.tile([128, NBLK, 27], F32)
        nc.sync.dma_start(FB_sb[:], d_FB.ap())
        xpad_sb = p_head.tile([128, 2, PADHW], BF)
        nc.sync.dma_start(
            xpad_sb[:],
            _ap(d_xpad.ap(), 0, [[PADHW, 128], [128 * PADHW, 2], [1, PADHW]]))

        # ---------------- offset conv + om transpose (per N-tile) ----------------
        omT = p_head.tile([128, NBLK, 27], F32)
        for nt in range(8):
            ps = p_ps_om.tile([27, 512], F32, tag="psom", name="psom")
            first = True
            for k in range(9):
                ky, kx = k // 3, k % 3
                for cb in range(2):
                    rhs = _ap(xpad_sb[:], cb * PADHW + (nt * 8 + ky) * 66 + kx,
                              [[2 * PADHW, 128], [66, 8], [1, 64]])
                    nc.tensor.matmul(ps[:], woff_sb[:, k * 2 + cb, :], rhs,
                                     start=first, stop=(k == 8 and cb == 1))
                    first = False
            om_nt = p_om.tile([27, 512], F32, tag="omnt", name="omnt")
            nc.scalar.copy(om_nt[:], ps[:])
            for i in range(4):
                pst = p_ps_om.tile([128, 27], F32, tag="pstr", name="pstr")
                nc.tensor.transpose(pst[:], om_nt[:, i * 128:(i + 1) * 128],
                                    id27_sb[:])
                nc.scalar.copy(omT[:, nt * 4 + i, :], pst[:])

        # ---------------- fields ----------------
        NF = NBLK * 9  # 288

        def f9(tt, j0):  # [128, NBLK, 9] view at channel offset j0
            return _ap(tt[:], j0, [[NBLK * 27, 128], [27, NBLK], [1, 9]])

        def ftile(tag, dt=F32):
            return p_ftmp.tile([128, NF], dt, tag=tag, name=tag)

        px = ftile("px"); py = ftile("py"); mask = ftile("mask")
        x0 = ftile("x0"); y0 = ftile("y0"); ti32 = ftile("i32tmp", I32)
        cy0 = ftile("cy0"); cy1 = ftile("cy1"); xp = ftile("xp"); yb = ftile("yb")
        ta = ftile("ta"); tb = ftile("tb"); tc_ = ftile("tc"); td = ftile("td")
        cx0 = ftile("cx0"); cx1 = ftile("cx1")
        yb_bf = p_ftmp.tile([128, NF], BF, tag="ybbf", name="ybbf")
        xp_bf = p_ftmp.tile([128, NF], BF, tag="xpbf", name="xpbf")

        nc.vector.tensor_tensor(px[:], f9(omT, 0), f9(FB_sb, 0), AL.add)
        nc.vector.tensor_tensor(py[:], f9(omT, 9), f9(FB_sb, 9), AL.add)
        nc.vector.tensor_tensor(mask[:], f9(omT, 18), f9(FB_sb, 18), AL.add)
        nc.scalar.activation(mask[:], mask[:], AF.Sigmoid)

        # floor, robust to cast rounding mode: f = cast(v); f -= (f > v)
        nc.vector.tensor_copy(ti32[:], px[:])
        nc.vector.tensor_copy(x0[:], ti32[:])
        nc.vector.tensor_tensor(ta[:], x0[:], px[:], AL.is_gt)
        nc.vector.tensor_tensor(x0[:], x0[:], ta[:], AL.subtract)
        nc.vector.tensor_copy(ti32[:], py[:])
        nc.vector.tensor_copy(y0[:], ti32[:])
        nc.vector.tensor_tensor(ta[:], y0[:], py[:], AL.is_gt)
        nc.vector.tensor_tensor(y0[:], y0[:], ta[:], AL.subtract)

        # px/py become fx/fy in place
        nc.vector.tensor_tensor(px[:], px[:], x0[:], AL.subtract)
        nc.vector.tensor_tensor(py[:], py[:], y0[:], AL.subtract)
        fx, fy = px, py

        # y slots: one chunk covers rows (yb, yb+1); yb = clip(y0,0,63).
        # wy0m = (1-fy)*vy0*mask ; wy1m = fy*vy1*mask
        # cy0 (row yb)   = wy0m + wy1m*[y0 == -1]
        # cy1 (row yb+1) = wy1m * [y0 >= 0]
        nc.vector.tensor_scalar(ta[:], y0[:], 0.0, None, AL.is_ge)
        nc.vector.tensor_scalar(tb[:], y0[:], 63.0, None, AL.is_le)
        nc.vector.tensor_tensor(ta[:], ta[:], tb[:], AL.mult)
        nc.vector.tensor_tensor(ta[:], ta[:], mask[:], AL.mult)
        nc.vector.tensor_scalar(tb[:], fy[:], -1.0, 1.0, AL.mult, AL.add)
        nc.vector.tensor_tensor(cy0[:], tb[:], ta[:], AL.mult)   # wy0m
        nc.vector.tensor_scalar(ta[:], y0[:], -1.0, None, AL.is_ge)
        nc.vector.tensor_scalar(tb[:], y0[:], 62.0, None, AL.is_le)
        nc.vector.tensor_tensor(ta[:], ta[:], tb[:], AL.mult)
        nc.vector.tensor_tensor(ta[:], ta[:], mask[:], AL.mult)
        nc.vector.tensor_tensor(cy1[:], fy[:], ta[:], AL.mult)   # wy1m
        nc.vector.tensor_scalar(ta[:], y0[:], -1.0, None, AL.is_equal)
        nc.vector.tensor_tensor(ta[:], ta[:], cy1[:], AL.mult)
        nc.vector.tensor_tensor(cy0[:], cy0[:], ta[:], AL.add)
        nc.vector.tensor_scalar(ta[:], y0[:], 0.0, None, AL.is_ge)
        nc.vector.tensor_tensor(cy1[:], cy1[:], ta[:], AL.mult)
        nc.vector.tensor_scalar(yb[:], y0[:], 0.0, 63.0, AL.max, AL.min)

        # x slots
        nc.vector.tensor_scalar(xp[:], x0[:], 0.0, 62.0, AL.max, AL.min)
        nc.vector.tensor_tensor(ta[:], x0[:], xp[:], AL.subtract)      # s
        nc.vector.tensor_scalar(tb[:], x0[:], 0.0, None, AL.is_ge)
        nc.vector.tensor_scalar(tc_[:], x0[:], 63.0, None, AL.is_le)
        nc.vector.tensor_tensor(tb[:], tb[:], tc_[:], AL.mult)         # vx0
        nc.vector.tensor_scalar(tc_[:], fx[:], -1.0, 1.0, AL.mult, AL.add)
        nc.vector.tensor_tensor(tc_[:], tc_[:], tb[:], AL.mult)        # wx0
        nc.vector.tensor_scalar(tb[:], x0[:], -1.0, None, AL.is_ge)
        nc.vector.tensor_scalar(td[:], x0[:], 62.0, None, AL.is_le)
        nc.vector.tensor_tensor(tb[:], tb[:], td[:], AL.mult)          # vx1
        nc.vector.tensor_tensor(td[:], fx[:], tb[:], AL.mult)          # wx1

        nc.vector.tensor_scalar(tb[:], ta[:], 0.0, None, AL.is_equal)
        nc.vector.tensor_tensor(cx0[:], tb[:], tc_[:], AL.mult)
        nc.vector.tensor_tensor(cx1[:], tb[:], td[:], AL.mult)
        nc.vector.tensor_scalar(tb[:], ta[:], -1.0, None, AL.is_equal)
        nc.vector.tensor_tensor(tb[:], tb[:], td[:], AL.mult)
        nc.vector.tensor_tensor(cx0[:], cx0[:], tb[:], AL.add)
        nc.vector.tensor_scalar(tb[:], ta[:], 1.0, None, AL.is_equal)
        nc.vector.tensor_tensor(tb[:], tb[:], tc_[:], AL.mult)
        nc.vector.tensor_tensor(cx1[:], cx1[:], tb[:], AL.add)

        # coefP: each coef stored twice (dup pairs) so pass1 can read
        # 4B-aligned step-1 pairs: col = blk*72 + (k*4 + xs*2 + yc)*2 + dup
        def coef_ap(xs, yc, dup):
            return _ap(coefP[:], (xs * 2 + yc) * 2 + dup,
                       [[NBLK * 72, 128], [72, NBLK], [8, 9]])

        for (cxv, cyv, xs, yc) in ((cx0, cy0, 0, 0), (cx0, cy1, 0, 1),
                                   (cx1, cy0, 1, 0), (cx1, cy1, 1, 1)):
            nc.vector.tensor_tensor(tb[:], cxv[:], cyv[:], AL.mult)
            nc.vector.tensor_copy(coef_ap(xs, yc, 0), tb[:])
            nc.vector.tensor_copy(coef_ap(xs, yc, 1), tb[:])

        # ---------------- on-chip gather index build ----------------
        # gidx [128, blk(32), k(9)*8+g] int16, idx = 64*yb + xp, wrapped-16
        # layout replicated across the 8 partition groups via selection
        # matmuls: out_g[i, c] = 64*yb[g*16 + i%16, c] + xp[g*16 + i%16, c].
        nc.vector.tensor_copy(yb_bf[:], yb[:])
        nc.vector.tensor_copy(xp_bf[:], xp[:])
        for g in range(8):
            psi = p_ps_om.tile([128, NF], F32, tag="psom", name="psidx")
            nc.tensor.matmul(psi[:], sel_sb[:, g, 0, :], yb_bf[:],
                             start=True, stop=False)
            nc.tensor.matmul(psi[:], sel_sb[:, g, 1, :], xp_bf[:],
                             start=False, stop=True)
            nc.vector.tensor_copy(
                _ap(gidx[:], g, [[NBLK * 72, 128], [72, NBLK], [8, 9]]),
                psi[:])

        # free head-phase SBUF/PSUM before the main loop
        hctx.close()

        p_G = ctx.enter_context(tc.tile_pool(name="G", bufs=5))
        p_gT = ctx.enter_context(tc.tile_pool(name="gT", bufs=2))
        p_gall = ctx.enter_context(tc.tile_pool(name="gall", bufs=2))
        p_ps_dcn = ctx.enter_context(tc.tile_pool(name="psdcn", bufs=2, space="PSUM"))
        p_ps_dc = ctx.enter_context(tc.tile_pool(name="psdc", bufs=2, space="PSUM"))
        p_outst = ctx.enter_context(tc.tile_pool(name="outst", bufs=2))

        xT2_src = _ap(d_xT2.ap(), 0, [[512, HW + 1], [1, 1024]])

        # ---------------- main loop over 128-position blocks ----------------
        for h in range(NHT):
            gall = p_gall.tile([128, 2, 18, 128], BF, tag="gall", name="gall")
            for pb in range(2):
                blk = h * 2 + pb
                G = p_G.tile([128, 9, 1024], BF, tag="G", name="G")
                nc.gpsimd.dma_gather(
                    G[:], xT2_src,
                    _ap(gidx[:], blk * 72, [[NBLK * 72, 128], [1, 72]]),
                    1152, 1152, 1024, elem_step=512,
                    queue_num=blk % 4, single_packet=False)
                # pass1 (in-place): G *= coef broadcast over channels
                g_view = _ap(G[:], 0,
                             [[9 * 1024, 128], [256, 36], [1, 256]])
                c_view = _ap(coefP[:], blk * 72,
                             [[NBLK * 72, 128], [2, 36], [0, 256]])
                nc.vector.tensor_tensor(g_view, g_view, c_view, AL.mult)
                # pass2: yc pairs into yc0 slots
                ev = _ap(G[:], 0, [[9 * 1024, 128], [512, 18], [1, 256]])
                od = _ap(G[:], 256, [[9 * 1024, 128], [512, 18], [1, 256]])
                nc.vector.tensor_tensor(ev, ev, od, AL.add)
                # pass3: xs pairs -> gT [128, k(9), 256]
                gT = p_gT.tile([128, 9 * 256], BF, tag="gT", name="gT")
                p3o = _ap(gT[:], 0, [[9 * 256, 128], [256, 9], [1, 256]])
                p3a = _ap(G[:], 0, [[9 * 1024, 128], [1024, 9], [1, 256]])
                p3b = _ap(G[:], 512, [[9 * 1024, 128], [1024, 9], [1, 256]])
                nc.vector.tensor_tensor(p3o, p3a, p3b, AL.add)
                # gT holds 9 k-chunks (one pb) = [128, 2304]; contiguous dest
                nc.sync.dma_start_transpose(
                    _ap(gall[:], pb * 2304,
                        [[2 * 18 * 128, 128], [128, 18], [1, 128]]),
                    gT[:])
            # DCN matmul + BN1+ReLU into bands
            for ohalf in range(2):
                ps = p_ps_dcn.tile([128, 256], F32, tag="psdcn", name="psdcn")
                for j in range(18):
                    lhsT = _ap(wdcn_sb[:], j * 256 + ohalf * 128,
                               [[18 * 256, 128], [1, 128]])
                    rhs = _ap(gall[:], j * 128,
                              [[2 * 18 * 128, 128], [2304, 2], [1, 128]])
                    nc.tensor.matmul(ps[:], lhsT, rhs,
                                     start=(j == 0), stop=(j == 17))
                td0 = h // 2
                loc0 = 4 * (h % 2) + 1
                bb = bn1_sb[:, 1, ohalf:ohalf + 1]
                ss = bn1_sb[:, 0, ohalf:ohalf + 1]
                nc.scalar.activation(
                    _ap(bands[:], td0 * 1320 + ohalf * 660 + loc0 * 66 + 1,
                        [[8 * 2 * 660, 128], [66, 4], [1, 64]]),
                    ps[:], AF.Relu, bias=bb, scale=ss)
                if h % 2 == 0 and td0 > 0:
                    nc.scalar.activation(
                        _ap(bands[:], (td0 - 1) * 1320 + ohalf * 660 + 9 * 66 + 1,
                            [[8 * 2 * 660, 128], [1, 64]]),
                        ps[:, 0:64], AF.Relu, bias=bb, scale=ss)
                if h % 2 == 1 and td0 < 7:
                    nc.scalar.activation(
                        _ap(bands[:], (td0 + 1) * 1320 + ohalf * 660 + 1,
                            [[8 * 2 * 660, 128], [1, 64]]),
                        ps[:, 192:256], AF.Relu, bias=bb, scale=ss)

            # deconv for ready band
            td_ = None
            if h >= 2 and h % 2 == 0:
                td_ = h // 2 - 1
            elif h == NHT - 1:
                td_ = 7
            if td_ is None:
                continue
            for ohalf in range(2):
                outst = p_outst.tile([128, 2048], F32, tag="outst", name="outst")
                for par in range(4):
                    a, b_ = par // 2, par % 2
                    tap_y = TAP0 if a == 0 else TAP1
                    tap_x = TAP0 if b_ == 0 else TAP1
                    ps = p_ps_dc.tile([128, 512], F32, tag="psdc", name="psdc")
                    for j8 in range(8):
                        ti, tj, chalf = j8 // 4, (j8 // 2) % 2, j8 % 2
                        dr, ds = tap_y[ti][1], tap_x[tj][1]
                        lhsT = _ap(wup_sb[:],
                                   par * 8 * 256 + j8 * 256 + ohalf * 128,
                                   [[4 * 8 * 256, 128], [1, 128]])
                        rhs = _ap(bands[:],
                                  td_ * 1320 + chalf * 660 + (1 + dr) * 66 + 1 + ds,
                                  [[8 * 2 * 660, 128], [66, 8], [1, 64]])
                        nc.tensor.matmul(ps[:], lhsT, rhs,
                                         start=(j8 == 0), stop=(j8 == 7))
                    nc.scalar.activation(
                        _ap(outst[:], a * 128 + b_,
                            [[2048, 128], [256, 8], [2, 64]]),
                        ps[:], AF.Relu,
                        bias=bn2_sb[:, 1, ohalf:ohalf + 1], scale=bn2_sb[:, 0, ohalf:ohalf + 1])
                nc.sync.dma_start(
                    _ap(d_out.ap(), ohalf * 128 * 16384 + td_ * 16 * 128,
                        [[16384, 128], [1, 2048]]),
                    outst[:])

    nc.compile()
    return nc


# ---------------- host prep ----------------
def _prep_shared(inputs):
    w_off = np.asarray(inputs["w_off"], np.float32)
    b_off = np.asarray(inputs["b_off"], np.float32)
    w_dcn = np.asarray(inputs["w_dcn"], np.float32)
    w_up = np.asarray(inputs["w_up"], np.float32)

    woff = np.zeros((9, 2, 128, 27), np.float32)
    for k in range(9):
        for cb in range(2):
            woff[k, cb] = w_off[:, cb * 128:(cb + 1) * 128, k // 3, k % 3].T
    wdcn = np.zeros((128, 18, 256), np.float32)
    wd = w_dcn.reshape(Co, C, 9)
    for k in range(9):
        for chalf in range(2):
            wdcn[:, k * 2 + chalf, :] = wd[:, chalf * 128:(chalf + 1) * 128, k].T
    wup = np.zeros((128, 4, 8, 256), np.float32)
    for par in range(4):
        a, b_ = par // 2, par % 2
        tap_y = TAP0 if a == 0 else TAP1
        tap_x = TAP0 if b_ == 0 else TAP1
        for j8 in range(8):
            ti, tj, chalf = j8 // 4, (j8 // 2) % 2, j8 % 2
            kh, kw = tap_y[ti][0], tap_x[tj][0]
            # lhsT[p=c%128, o] = w_eff[o, c] = w_up[o, c, kh, kw]
            wup[:, par, j8, :] = w_up[:, chalf * 128:(chalf + 1) * 128, kh, kw].T

    ky = np.repeat(np.arange(3) - 1, 3).astype(np.float32)
    kx = np.tile(np.arange(3) - 1, 3).astype(np.float32)
    pos = np.arange(HW)
    hh = (pos // W).astype(np.float32)
    ww = (pos % W).astype(np.float32)
    FBp = np.zeros((HW, 27), np.float32)
    FBp[:, 0:9] = ww[:, None] + kx[None, :]
    FBp[:, 9:18] = hh[:, None] + ky[None, :]
    FBp += b_off[None, :]
    FB = np.ascontiguousarray(FBp.reshape(NBLK, 128, 27).transpose(1, 0, 2))

    # selection weights for the on-chip index fold:
    # sel[q, g, 0, i] = 64 if q == g*16 + i%16 ; sel[q, g, 1, i] = 1 if same
    sel = np.zeros((128, 8, 2, 128), np.float32)
    q = np.arange(128)
    i = np.arange(128)
    for g in range(8):
        m = (q[:, None] == g * 16 + (i[None, :] % 16))
        sel[:, g, 0, :] = 64.0 * m
        sel[:, g, 1, :] = 1.0 * m

    def bnfold(g, b, m, v):
        s = np.asarray(g) / np.sqrt(np.asarray(v) + EPS)
        return s.astype(np.float32), (np.asarray(b) - np.asarray(m) * s).astype(np.float32)

    s1, b1 = bnfold(inputs["bn1_g"], inputs["bn1_b"], inputs["bn1_m"], inputs["bn1_v"])
    s2, b2 = bnfold(inputs["bn2_g"], inputs["bn2_b"], inputs["bn2_m"], inputs["bn2_v"])
    bn1 = np.stack([s1.reshape(2, 128), b1.reshape(2, 128)])
    bn2 = np.stack([s2.reshape(2, 128), b2.reshape(2, 128)])

    return dict(
        woff=woff.astype(BF16), wdcn=wdcn.astype(BF16), wup=wup.astype(BF16),
        FB=FB.astype(np.float32), bn1=bn1.astype(np.float32),
        bn2=bn2.astype(np.float32), id27=np.eye(27, dtype=np.float32),
        sel=sel.astype(BF16),
    )


def _prep_sample(xb):
    xb = np.asarray(xb, np.float32)
    xpad = np.zeros((C, 66, 66), np.float32)
    xpad[:, 1:65, 1:65] = xb.reshape(C, 64, 64)
    xpad = xpad.reshape(2, 128, PADHW)
    # xT2[p] = [ch(y,x), ch(y+1,x)] for p = y*64+x; zeros for y+1 == 64
    xT = xb.reshape(C, HW).T          # [HW, 256]
    xT2 = np.zeros((HW + 2, 512), np.float32)
    xT2[:HW, 0:256] = xT
    xT2[:HW - 64, 256:512] = xT[64:]
    return dict(xpad=xpad.astype(BF16), xT2=xT2.reshape(-1).astype(BF16))


_NC_CACHE = {}
TRACE = False
LAST_RESULT = None


def kernel(**inputs):
    global LAST_RESULT
    if "nc" not in _NC_CACHE:
        _NC_CACHE["nc"] = build_nc()
    nc = _NC_CACHE["nc"]
    shared = _prep_shared(inputs)
    x = np.asarray(inputs["x"])
    in_maps = [dict(shared, **_prep_sample(x[b])) for b in range(x.shape[0])]
    res = bass_utils.run_bass_kernel_spmd(nc, in_maps, core_ids=list(range(8)),
                                          trace=TRACE)
    LAST_RESULT = res
    out = np.stack([res.results[b]["out"] for b in range(len(in_maps))])
    return out.astype(np.float32)
